# revision 9
# baseline (speedup 1.0000x reference)
"""Trainium2 Bass kernel for nn_CosineLoss: mean_i(1 - output[i, targets[i]]).

Strategy (data-parallel over the batch dim, 8 cores):
  - Core c owns rows [c*1024, (c+1)*1024) of `output` ([1024, 32000] f32 shard)
    plus flat element offsets idx[i] = i*32000 + targets[i] for its rows
    (int32, laid out [128, 8] in SBUF).
  - On device: 8 indirect DMAs (128 descriptors each -- the HW unrolls one
    descriptor per dest partition row and consumes ONE offset per row, so 128
    scattered elements per instruction is a hard cap; probed on HW) gather
    the 1024 needed f32 elements from HBM (4 KB instead of 131 MB).
    Q7 descriptor generation is the serial bottleneck: ~1.4us per SWDGE
    instruction (994ns fixed ucode prologue + dispatch), ~11us total.
    (InstDMAGatherAnt would do 512 offsets per instruction but its ext-isa
    ucode measures ~4.6us/call plus an ~8.5us one-time IRAM library load --
    strictly worse. Measured, not guessed.)
  - Output path: gpsimd issues a 9th SWDGE DMA right after the gathers that
    reads the gathered [128, 8] tile back to DRAM. Same qPoolDynamic ring
    and same partition->engine mapping as the gathers' SBUF writes, so each
    SDMA engine drains it strictly after its gather descriptors -- no
    completion semaphore, no DVE/PE/ACT chain on the critical path
    (saves ~1.2us of post-gather latency vs reduce-then-DMA).
  - Host sums the 8x1024 gathered values and returns 1 - total/8192.
"""

import numpy as np

from concourse import bacc, bass, mybir
from concourse.bass_utils import run_bass_kernel_spmd

N = 8192
C = 32000
NCORES = 8
NL = N // NCORES  # 1024 rows per core
P = 128
F = NL // P  # 8 gathered elements per partition

_NC_CACHE = {}


def _build():
    nc = bacc.Bacc("TRN2")
    x = nc.dram_tensor("x", [NL, C], mybir.dt.float32, kind="ExternalInput")
    idx = nc.dram_tensor("idx", [P, F], mybir.dt.int32, kind="ExternalInput")
    gout = nc.dram_tensor("gout", [P, F], mybir.dt.float32, kind="ExternalOutput")

    idx_t = nc.alloc_sbuf_tensor("idx_t", [P, F], mybir.dt.int32)
    gath = nc.alloc_sbuf_tensor("gath", [P, F], mybir.dt.float32)

    s_idx = nc.alloc_semaphore("s_idx")  # idx DMA completion (+16)
    s_g = nc.alloc_semaphore("s_g")  # gather DMA completions (+16 each)
    s_out = nc.alloc_semaphore("s_out")  # readback completion (unwaited; exit drain covers it)

    # idx on the Scalar HWDGE: Scalar finishes its kernel-entry code ~0.8us
    # before Sync (whose entry includes a 703ns queue DRAIN), so the
    # idx->gather dependency chain starts that much earlier. Also measured
    # faster than a gpsimd SWDGE load (22629 ns) -- Q7 gen occupancy
    # outweighs the HWDGE pipe's longer first-byte latency.
    nc.scalar.dma_start(out=idx_t.ap(), in_=idx[:]).then_inc(s_idx, 16)

    nc.gpsimd.wait_ge(s_idx, 16)
    for j in range(F):
        nc.gpsimd.indirect_dma_start(
            out=gath.ap()[:, j : j + 1],
            out_offset=None,
            in_=x[:],
            in_offset=bass.IndirectOffsetOnAxis(ap=idx_t.ap()[:, j : j + 1], axis=1),
        ).then_inc(s_g, 16)

    # ring-ordered readback: descriptors queue behind the gathers on the
    # same per-engine FIFOs, so this needs no wait on s_g.
    nc.gpsimd.dma_start(out=gout[:], in_=gath.ap()).then_inc(s_out, 16)

    nc.compile()
    return nc


def _get_nc():
    if "nc" not in _NC_CACHE:
        _NC_CACHE["nc"] = _build()
    return _NC_CACHE["nc"]


def _shard(output, targets):
    xs = np.ascontiguousarray(
        output.reshape(NCORES, NL, C).astype(np.float32, copy=False)
    )
    flat = np.arange(NL, dtype=np.int32) * C + targets.reshape(NCORES, NL).astype(
        np.int32
    )
    return xs, np.ascontiguousarray(flat.reshape(NCORES, P, F))


def _run(output, targets, **kwargs):
    xs, idx = _shard(output, targets)
    in_maps = [{"x": xs[c], "idx": idx[c]} for c in range(NCORES)]
    return run_bass_kernel_spmd(
        _get_nc(), in_maps, core_ids=list(range(NCORES)), **kwargs
    )


def kernel(output, targets):
    res = _run(output, targets)
    total = sum(float(r["gout"].sum(dtype=np.float64)) for r in res.results)
    return np.array(np.float32(1.0) - np.float32(total / N), dtype=np.float32)


# revision 10
# speedup vs baseline: 1.0296x; 1.0296x over previous
"""Trainium2 Bass kernel for nn_CosineLoss: mean_i(1 - output[i, targets[i]]).

Strategy (data-parallel over the batch dim, 8 cores):
  - Core c owns rows [c*1024, (c+1)*1024) of `output` ([1024, 32000] f32 shard)
    plus flat element offsets idx[i] = i*32000 + targets[i] for its rows
    (int32, laid out [128, 8] in SBUF).
  - On device: 8 indirect DMAs (128 descriptors each -- the HW unrolls one
    descriptor per dest partition row and consumes ONE offset per row, so 128
    scattered elements per instruction is a hard cap; probed on HW) gather
    the 1024 needed f32 elements from HBM (4 KB instead of 131 MB).
    Q7 descriptor generation is the serial bottleneck: ~1.4us per SWDGE
    instruction (994ns fixed ucode prologue + dispatch), ~11us total.
    (InstDMAGatherAnt would do 512 offsets per instruction but its ext-isa
    ucode measures ~4.6us/call plus an ~8.5us one-time IRAM library load --
    strictly worse. Measured, not guessed.)
  - Output path: gpsimd issues a 9th SWDGE DMA right after the gathers that
    reads the gathered [128, 8] tile back to DRAM. Same qPoolDynamic ring
    and same partition->engine mapping as the gathers' SBUF writes, so each
    SDMA engine drains it strictly after its gather descriptors -- no
    completion semaphore, no DVE/PE/ACT chain on the critical path
    (saves ~1.2us of post-gather latency vs reduce-then-DMA).
  - Host sums the 8x1024 gathered values and returns 1 - total/8192.
"""

import numpy as np

from concourse import bacc, bass, mybir
from concourse.bass_utils import run_bass_kernel_spmd

N = 8192
C = 32000
NCORES = 8
NL = N // NCORES  # 1024 rows per core
P = 128
F = NL // P  # 8 gathered elements per partition

_NC_CACHE = {}


def _build():
    nc = bacc.Bacc("TRN2")
    x = nc.dram_tensor("x", [NL, C], mybir.dt.float32, kind="ExternalInput")
    idx = nc.dram_tensor("idx", [P, F], mybir.dt.int32, kind="ExternalInput")
    gout = nc.dram_tensor("gout", [P, F], mybir.dt.float32, kind="ExternalOutput")

    idx_t = nc.alloc_sbuf_tensor("idx_t", [P, F], mybir.dt.int32)
    gath = nc.alloc_sbuf_tensor("gath", [P, F], mybir.dt.float32)

    s_idx = nc.alloc_semaphore("s_idx")  # idx DMA completion (+16)
    s_g = nc.alloc_semaphore("s_g")  # gather DMA completions (+16 each)
    s_out = nc.alloc_semaphore("s_out")  # readback completion (unwaited; exit drain covers it)

    # idx on the Sync HWDGE. Engine choice is immaterial (Scalar measured
    # identical): the bass entry all_engine_barrier gates every engine's
    # first instruction on Sync's ~703ns queue DRAIN, so the first HWDGE
    # DMA can't issue before ~7.0us regardless. A gpsimd SWDGE idx load
    # measured worse (22629 ns): Q7 gen occupancy outweighs the HWDGE
    # pipe's longer first-byte latency.
    nc.sync.dma_start(out=idx_t.ap(), in_=idx[:]).then_inc(s_idx, 16)

    nc.gpsimd.wait_ge(s_idx, 16)
    for j in range(F):
        nc.gpsimd.indirect_dma_start(
            out=gath.ap()[:, j : j + 1],
            out_offset=None,
            in_=x[:],
            in_offset=bass.IndirectOffsetOnAxis(ap=idx_t.ap()[:, j : j + 1], axis=1),
        ).then_inc(s_g, 16)

    # ring-ordered readback: descriptors queue behind the gathers on the
    # same per-engine FIFOs, so this needs no wait on s_g.
    nc.gpsimd.dma_start(out=gout[:], in_=gath.ap()).then_inc(s_out, 16)

    nc.compile()
    return nc


def _get_nc():
    if "nc" not in _NC_CACHE:
        _NC_CACHE["nc"] = _build()
    return _NC_CACHE["nc"]


def _shard(output, targets):
    xs = np.ascontiguousarray(
        output.reshape(NCORES, NL, C).astype(np.float32, copy=False)
    )
    flat = np.arange(NL, dtype=np.int32) * C + targets.reshape(NCORES, NL).astype(
        np.int32
    )
    return xs, np.ascontiguousarray(flat.reshape(NCORES, P, F))


def _run(output, targets, **kwargs):
    xs, idx = _shard(output, targets)
    in_maps = [{"x": xs[c], "idx": idx[c]} for c in range(NCORES)]
    return run_bass_kernel_spmd(
        _get_nc(), in_maps, core_ids=list(range(NCORES)), **kwargs
    )


def kernel(output, targets):
    res = _run(output, targets)
    total = sum(float(r["gout"].sum(dtype=np.float64)) for r in res.results)
    return np.array(np.float32(1.0) - np.float32(total / N), dtype=np.float32)
